# revision 1
# baseline (speedup 1.0000x reference)
"""Trainium2 Bass kernel for nn_LSTMDiscriminator.

LSTM (L=512, B=1024, X=128, H=256) + 3-layer MLP head, data-parallel over
batch across 8 NeuronCores (128 samples per core).

Per-core layout (per timestep):
  gates G[B=128, 4H=1024] accumulate in PSUM from 3 matmuls
  (x_t^T and h^T are the stationary operands; weights stream).
  Host pre-permutes gates to (i,f,o,g) and scales the g rows by 2 so a
  single Sigmoid activation covers all gates; tanh(z) = 2*sigmoid(2z)-1.
  h is carried transposed (hT[H-chunk, B]) so it feeds the next step's
  matmul directly; the transpose runs on the PE (via identity matmul).
"""

import sys
import time

sys.path.insert(0, "/opt/trn_rl_repo")

import json
import numpy as np

import concourse.bass as bass
import concourse.tile as tile
from concourse import mybir
from concourse import bass2jax
from concourse.masks import make_identity

L, B, X, H = 512, 1024, 128, 256
NCORES = 8
BC = B // NCORES  # 128 per core
G4 = 4 * H  # 1024
F32 = mybir.dt.float32
AF = mybir.ActivationFunctionType
ALU = mybir.AluOpType

# ---------------------------------------------------------------------------
# Workaround: this walrus build accepts only ONE sync-wait per instruction.
# Split any instruction with N>1 on_wait conditions into N-1 single-wait
# NoOp carriers (same engine, program order preserved) + the instruction.
# ---------------------------------------------------------------------------


def _split_multi_waits(bir: dict) -> int:
    n_split = 0
    for fn in bir.get("functions", []):
        for blk in fn.get("blocks", []):
            out = []
            for inst in blk.get("instructions", []):
                si = inst.get("sync_info")
                waits = (si or {}).get("on_wait") or []
                if len(waits) > 1:
                    for k, w in enumerate(waits[:-1]):
                        out.append(
                            {
                                "debug": inst.get("debug", 0),
                                "engine": inst.get("engine"),
                                "ins": [],
                                "name": f"{inst['name']}-ws{k}",
                                "opcode": "NoOp",
                                "outs": [],
                                "sync_info": {"on_update": [], "on_wait": [w]},
                            }
                        )
                    si["on_wait"] = [waits[-1]]
                    n_split += 1
                out.append(inst)
            blk["instructions"] = out
    return n_split


def _install_fixup():
    from concourse import bass_utils

    if getattr(bass_utils, "_lstm_fixup_installed", False):
        return
    orig = bass_utils.compile_bir_kernel

    def wrapper(ant_bir_str, compile_dir_path, neff_name="file.neff", **kw):
        bir = json.loads(ant_bir_str)
        _split_multi_waits(bir)
        return orig(json.dumps(bir).encode(), compile_dir_path, neff_name=neff_name, **kw)

    bass_utils.compile_bir_kernel = wrapper
    bass_utils._lstm_fixup_installed = True
    bass2jax.compile_bir_kernel = wrapper


def _bcast(ap, n):
    """View a [1, m] DRAM AP as [n, m] via zero partition stride."""
    return bass.AP(tensor=ap.tensor, offset=ap.offset, ap=[[0, n]] + list(ap.ap[1:]))


# ---------------------------------------------------------------------------
# Kernel build
# ---------------------------------------------------------------------------


def _build():
    nc = bass.Bass("TRN2", target_bir_lowering=False, debug=False, num_devices=NCORES)
    xd = nc.dram_tensor("x", [L, BC, X], F32, kind="ExternalInput").ap()
    wihT = nc.dram_tensor("wihT", [X, G4], F32, kind="ExternalInput").ap()
    whhT = nc.dram_tensor("whhT", [H, G4], F32, kind="ExternalInput").ap()
    biasd = nc.dram_tensor("bias", [1, G4], F32, kind="ExternalInput").ap()
    w0T = nc.dram_tensor("w0T", [H, H], F32, kind="ExternalInput").ap()
    b0d = nc.dram_tensor("b0", [1, H], F32, kind="ExternalInput").ap()
    w1T = nc.dram_tensor("w1T", [H, H], F32, kind="ExternalInput").ap()
    b1d = nc.dram_tensor("b1", [1, H], F32, kind="ExternalInput").ap()
    w2T = nc.dram_tensor("w2T", [H, 1], F32, kind="ExternalInput").ap()
    b2d = nc.dram_tensor("b2", [1, 1], F32, kind="ExternalInput").ap()
    outd = nc.dram_tensor("out", [BC, 1], F32, kind="ExternalOutput").ap()

    with tile.TileContext(nc) as tc:
        with (
            tc.tile_pool(name="consts", bufs=1) as cp,
            tc.tile_pool(name="xin", bufs=6) as xp,
            tc.tile_pool(name="xtr", bufs=4) as xtp,
            tc.tile_pool(name="gact", bufs=2) as gp,
            tc.tile_pool(name="small", bufs=2) as sp,
            tc.tile_pool(name="gps", bufs=2, space="PSUM") as pg,
            tc.tile_pool(name="tps", bufs=2, space="PSUM") as pt,
            tc.tile_pool(name="mps", bufs=1, space="PSUM") as pm,
        ):
            # ---- constants ----
            ident = cp.tile([128, 128], F32)
            make_identity(nc, ident)
            wih_sb = cp.tile([128, G4], F32)
            nc.sync.dma_start(out=wih_sb, in_=wihT)
            whh_sb = cp.tile([128, 2, G4], F32)
            nc.sync.dma_start(
                out=whh_sb, in_=whhT.rearrange("(k p) n -> p k n", p=128)
            )
            bias_sb = cp.tile([128, G4], F32)
            nc.sync.dma_start(out=bias_sb, in_=_bcast(biasd, 128))
            w0_sb = cp.tile([128, 2, H], F32)
            nc.sync.dma_start(out=w0_sb, in_=w0T.rearrange("(k p) n -> p k n", p=128))
            b0_sb = cp.tile([128, H], F32)
            nc.sync.dma_start(out=b0_sb, in_=_bcast(b0d, 128))
            w1_sb = cp.tile([128, 2, H], F32)
            nc.sync.dma_start(out=w1_sb, in_=w1T.rearrange("(k p) n -> p k n", p=128))
            b1_sb = cp.tile([128, H], F32)
            nc.sync.dma_start(out=b1_sb, in_=_bcast(b1d, 128))
            w2_sb = cp.tile([128, 2, 1], F32)
            nc.sync.dma_start(out=w2_sb, in_=w2T.rearrange("(k p) n -> p k n", p=128))
            b2_sb = cp.tile([128, 1], F32)
            nc.sync.dma_start(out=b2_sb, in_=_bcast(b2d, 128))

            # ---- initial state ----
            c_cur = sp.tile([128, H], F32, tag="c")
            nc.vector.memset(c_cur, 0.0)
            hT_cur = sp.tile([128, 2, BC], F32, tag="hT")
            nc.vector.memset(hT_cur, 0.0)

            def transpose128(dst_sb, src_sb):
                tp = pt.tile([128, 128], F32, tag="tp")
                nc.tensor.transpose(tp, src_sb, ident)
                nc.vector.tensor_copy(dst_sb, tp)

            # ---- recurrence ----
            for t in range(L):
                xt = xp.tile([128, X], F32, tag="xt")
                nc.sync.dma_start(out=xt, in_=xd[t])
                xtT = xtp.tile([128, BC], F32, tag="xtT")
                transpose128(xtT, xt)

                G = pg.tile([128, G4], F32, tag="G")
                for s in range(2):
                    sl = bass.ts(s, 512)
                    nc.tensor.matmul(
                        G[:, sl], xtT, wih_sb[:, sl], start=True, stop=False
                    )
                    nc.tensor.matmul(
                        G[:, sl],
                        hT_cur[:, 0, :],
                        whh_sb[:, 0, sl],
                        start=False,
                        stop=False,
                    )
                    nc.tensor.matmul(
                        G[:, sl],
                        hT_cur[:, 1, :],
                        whh_sb[:, 1, sl],
                        start=False,
                        stop=True,
                    )

                # gate layout: slice0 = {i [0:256], g [256:512]},
                #              slice1 = {f [512:768], o [768:1024]}
                # slice0's sigmoid + c-partials overlap slice1's matmuls/sigmoid
                glin0 = gp.tile([128, 512], F32, tag="glin0")
                nc.vector.tensor_add(glin0, G[:, 0:512], bias_sb[:, 0:512])
                sig0 = gp.tile([128, 512], F32, tag="sig0")
                nc.scalar.activation(sig0, glin0, AF.Sigmoid)
                tg = sp.tile([128, H], F32, tag="tg")  # tanh(g) = 2*sig(2g)-1
                nc.vector.tensor_scalar(tg, sig0[:, 256:512], 2.0, -1.0, ALU.mult, ALU.add)
                t2 = sp.tile([128, H], F32, tag="t2")
                nc.vector.tensor_mul(t2, sig0[:, 0:256], tg)

                glin1 = gp.tile([128, 512], F32, tag="glin1")
                nc.vector.tensor_add(glin1, G[:, 512:1024], bias_sb[:, 512:1024])
                sig1 = gp.tile([128, 512], F32, tag="sig1")
                nc.scalar.activation(sig1, glin1, AF.Sigmoid)
                t1 = sp.tile([128, H], F32, tag="t1")
                nc.vector.tensor_mul(t1, sig1[:, 0:256], c_cur)
                c_new = sp.tile([128, H], F32, tag="c")
                nc.vector.tensor_add(c_new, t1, t2)

                sigc = sp.tile([128, H], F32, tag="sigc")
                nc.scalar.activation(sigc, c_new, AF.Sigmoid, scale=2.0)
                tanc = sp.tile([128, H], F32, tag="tanc")
                nc.vector.tensor_scalar(tanc, sigc, 2.0, -1.0, ALU.mult, ALU.add)
                h_sb = sp.tile([128, H], F32, tag="h")
                nc.vector.tensor_mul(h_sb, sig1[:, 256:512], tanc)

                hT_new = sp.tile([128, 2, BC], F32, tag="hT")
                transpose128(hT_new[:, 0, :], h_sb[:, 0:128])
                transpose128(hT_new[:, 1, :], h_sb[:, 128:256])
                hT_cur = hT_new
                c_cur = c_new

            # ---- MLP head: leaky_relu(z) = max(z,0) + 0.2*min(z,0) ----
            def linear(hT_in, w_sb, b_sb, n_out):
                m = pm.tile([128, n_out], F32, tag="mlp_ps")
                nc.tensor.matmul(
                    m, hT_in[:, 0, :], w_sb[:, 0, :], start=True, stop=False
                )
                nc.tensor.matmul(
                    m, hT_in[:, 1, :], w_sb[:, 1, :], start=False, stop=True
                )
                z = sp.tile([128, n_out], F32, tag="mlp_z")
                nc.vector.tensor_add(z, m, b_sb[:, 0:n_out])
                return z

            def leaky(z, n_out):
                pos = sp.tile([128, n_out], F32, tag="mlp_pos")
                nc.vector.tensor_scalar_max(pos, z, 0.0)
                neg = sp.tile([128, n_out], F32, tag="mlp_neg")
                nc.vector.tensor_scalar(neg, z, 0.0, 0.2, ALU.min, ALU.mult)
                a = sp.tile([128, n_out], F32, tag="mlp_a")
                nc.vector.tensor_add(a, pos, neg)
                return a

            z0 = linear(hT_cur, w0_sb, b0_sb, H)
            a0 = leaky(z0, H)
            a0T = sp.tile([128, 2, BC], F32, tag="aT0")
            transpose128(a0T[:, 0, :], a0[:, 0:128])
            transpose128(a0T[:, 1, :], a0[:, 128:256])
            z1 = linear(a0T, w1_sb, b1_sb, H)
            a1 = leaky(z1, H)
            a1T = sp.tile([128, 2, BC], F32, tag="aT1")
            transpose128(a1T[:, 0, :], a1[:, 0:128])
            transpose128(a1T[:, 1, :], a1[:, 128:256])
            z2 = linear(a1T, w2_sb, b2_sb, 1)
            nc.sync.dma_start(out=outd, in_=z2)

    return nc


# ---------------------------------------------------------------------------
# Host-side driver with cached compiled executable
# ---------------------------------------------------------------------------

_CACHE = {}


def _get_exec():
    if "exec" in _CACHE:
        return _CACHE["exec"]
    _install_fixup()
    bass2jax.install_neuronx_cc_hook()
    import jax

    nc = _build()

    part_name = nc.partition_id_tensor.name if nc.partition_id_tensor else None
    in_names, out_names, out_avals, zero_shapes = [], [], [], []
    for alloc in nc.m.functions[0].allocations:
        if not isinstance(alloc, mybir.MemoryLocationSet):
            continue
        name = alloc.memorylocations[0].name
        if alloc.kind == "ExternalInput":
            if name != part_name:
                in_names.append(name)
        elif alloc.kind == "ExternalOutput":
            out_names.append(name)
            shape = tuple(alloc.tensor_shape)
            dtype = mybir.dt.np(alloc.dtype)
            out_avals.append(jax.core.ShapedArray(shape, dtype))
            zero_shapes.append((shape, dtype))
    n_params = len(in_names)
    n_outs = len(out_names)
    all_in_names = in_names + out_names
    if part_name is not None:
        all_in_names = all_in_names + [part_name]
    donate = tuple(range(n_params, n_params + n_outs))

    def _body(*args):
        operands = list(args)
        if part_name is not None:
            operands.append(bass2jax.partition_id_tensor())
        outs = bass2jax._bass_exec_p.bind(
            *operands,
            out_avals=tuple(out_avals),
            in_names=tuple(all_in_names),
            out_names=tuple(out_names),
            lowering_input_output_aliases=(),
            sim_require_finite=True,
            sim_require_nnan=True,
            nc=nc,
        )
        return tuple(outs)

    devices = jax.devices()[:NCORES]
    mesh = bass2jax.Mesh(np.asarray(devices), ("core",))
    spec = (bass2jax.PartitionSpec("core"),)
    sharded = jax.jit(
        bass2jax.shard_map(
            _body,
            mesh=mesh,
            in_specs=spec * (n_params + n_outs),
            out_specs=spec * n_outs,
            check_rep=False,
        ),
        donate_argnums=donate,
        keep_unused=True,
    )
    _CACHE["exec"] = (sharded, in_names, out_names, zero_shapes)
    return _CACHE["exec"]


def _prep_inputs(x, W_ih, W_hh, b_ih, b_hh, W0, b0, W1, b1, W2, b2):
    # reorder gates (i,f,g,o) -> (i,g,f,o); scale g rows by 2 (tanh trick)
    idx = np.concatenate(
        [
            np.arange(0, 256),      # i
            np.arange(512, 768),    # g
            np.arange(256, 512),    # f
            np.arange(768, 1024),   # o
        ]
    )
    gscale = np.ones((G4, 1), np.float32)
    gscale[256:512] = 2.0
    wih_p = (W_ih[idx] * gscale).astype(np.float32)
    whh_p = (W_hh[idx] * gscale).astype(np.float32)
    bias_p = (((b_ih + b_hh)[idx]) * gscale[:, 0]).astype(np.float32)

    per_core_common = {
        "wihT": np.ascontiguousarray(wih_p.T),
        "whhT": np.ascontiguousarray(whh_p.T),
        "bias": bias_p.reshape(1, G4),
        "w0T": np.ascontiguousarray(W0.T.astype(np.float32)),
        "b0": b0.reshape(1, H).astype(np.float32),
        "w1T": np.ascontiguousarray(W1.T.astype(np.float32)),
        "b1": b1.reshape(1, H).astype(np.float32),
        "w2T": np.ascontiguousarray(W2.T.astype(np.float32)),
        "b2": b2.reshape(1, 1).astype(np.float32),
    }
    in_maps = []
    for i in range(NCORES):
        m = dict(per_core_common)
        m["x"] = np.ascontiguousarray(x[:, i * BC : (i + 1) * BC, :]).astype(np.float32)
        in_maps.append(m)
    return in_maps


def _concat_inputs(in_maps, in_names):
    return [
        np.concatenate([np.asarray(in_maps[c][n]) for c in range(NCORES)], axis=0)
        for n in in_names
    ]


def _run_concat(concat_in):
    sharded, in_names, out_names, zero_shapes = _get_exec()
    zeros = [np.zeros((NCORES * s[0],) + s[1:], d) for s, d in zero_shapes]
    out_arrs = sharded(*concat_in, *zeros)
    return np.asarray(out_arrs[0])  # [8*BC, 1]


def kernel(**inputs) -> np.ndarray:
    sharded, in_names, out_names, zero_shapes = _get_exec()
    in_maps = _prep_inputs(**{k: np.asarray(v) for k, v in inputs.items()})
    concat_in = _concat_inputs(in_maps, in_names)
    out = _run_concat(concat_in)
    return out.reshape(B, 1).astype(np.float32)


def timed_run(inputs, iters=5):
    """Returns (best_seconds, output). Inputs transferred to device once."""
    import jax

    sharded, in_names, out_names, zero_shapes = _get_exec()
    in_maps = _prep_inputs(**{k: np.asarray(v) for k, v in inputs.items()})
    concat_in = _concat_inputs(in_maps, in_names)
    out = _run_concat(concat_in)  # compile + warm
    mesh = bass2jax.Mesh(np.asarray(jax.devices()[:NCORES]), ("core",))
    shd = jax.sharding.NamedSharding(mesh, bass2jax.PartitionSpec("core"))
    dev_in = [jax.device_put(a, shd) for a in concat_in]
    times = []
    for _ in range(iters):
        zeros = [np.zeros((NCORES * s[0],) + s[1:], d) for s, d in zero_shapes]
        t0 = time.perf_counter()
        r = sharded(*dev_in, *zeros)
        jax.block_until_ready(r)
        times.append(time.perf_counter() - t0)
    return min(times), out.reshape(B, 1)



# revision 22
# speedup vs baseline: 24.4455x; 24.4455x over previous
"""Trainium2 Bass kernel for nn_LSTMDiscriminator.

LSTM (L=512, B=1024, X=128, H=256) + 3-layer MLP head, data-parallel over
batch across 8 NeuronCores (128 samples per core).

v2 design (per core, batch-on-partitions layout G[B=128, 4H]):
  - all matmul operands bf16 (fp32 PSUM accumulate): 4x PE throughput vs fp32
  - x pre-transposed on host to [X, L*BC] and preloaded whole into SBUF
    (128 KiB/partition) -> zero in-loop DMA
  - gate bias enters PSUM via a K=1 ones-matmul, so activations read PSUM
    directly and DVE does no bias adds
  - gate order (i, f | o, g): sigmoid(i,f) after slice0 stops, tanh(g)
    right after slice1 stops (critical path), sigmoid(o) off-path
  - tail: transpose c_new on PE, tanh on the transposed tile, then
    hT = tanh(cT) * oT in one DVE op writing bf16 hT straight to SBUF
    (h is only ever carried transposed - no h transpose / copy)
"""

import sys
import time

sys.path.insert(0, "/opt/trn_rl_repo")

import json
import numpy as np

import concourse.bass as bass
import concourse.tile as tile
from concourse import mybir
from concourse import bass2jax
from concourse.masks import make_identity

L, B, X, H = 512, 1024, 128, 256
NCORES = 8
BC = B // NCORES  # 128 per core
G4 = 4 * H  # 1024
F32 = mybir.dt.float32
BF16 = mybir.dt.bfloat16
AF = mybir.ActivationFunctionType
ALU = mybir.AluOpType

XCHUNK = 64  # steps per x-preload DMA
S0 = slice(0, 512)      # gates i (0:256), f (256:512)
S1 = slice(512, 1024)   # gates o (512:768), g (768:1024)

# ---------------------------------------------------------------------------
# Workaround: this walrus build accepts only ONE sync-wait per instruction.
# Split any instruction with N>1 on_wait conditions into N-1 single-wait
# NoOp carriers (same engine, program order preserved) + the instruction.
# ---------------------------------------------------------------------------


def _split_multi_waits(bir: dict) -> int:
    n_split = 0
    for fn in bir.get("functions", []):
        for blk in fn.get("blocks", []):
            out = []
            for inst in blk.get("instructions", []):
                si = inst.get("sync_info")
                waits = (si or {}).get("on_wait") or []
                if len(waits) > 1:
                    for k, w in enumerate(waits[:-1]):
                        out.append(
                            {
                                "debug": inst.get("debug", 0),
                                "engine": inst.get("engine"),
                                "ins": [],
                                "name": f"{inst['name']}-ws{k}",
                                "opcode": "NoOp",
                                "outs": [],
                                "sync_info": {"on_update": [], "on_wait": [w]},
                            }
                        )
                    si["on_wait"] = [waits[-1]]
                    n_split += 1
                out.append(inst)
            blk["instructions"] = out
    return n_split


def _install_fixup():
    from concourse import bass_utils

    if getattr(bass_utils, "_lstm_fixup_installed", False):
        return
    orig = bass_utils.compile_bir_kernel

    def wrapper(ant_bir_str, compile_dir_path, neff_name="file.neff", **kw):
        bir = json.loads(ant_bir_str)
        _split_multi_waits(bir)
        return orig(json.dumps(bir).encode(), compile_dir_path, neff_name=neff_name, **kw)

    bass_utils.compile_bir_kernel = wrapper
    bass_utils._lstm_fixup_installed = True
    bass2jax.compile_bir_kernel = wrapper


def _bcast(ap, n):
    """View a [1, m] DRAM AP as [n, m] via zero partition stride."""
    return bass.AP(tensor=ap.tensor, offset=ap.offset, ap=[[0, n]] + list(ap.ap[1:]))


# ---------------------------------------------------------------------------
# Kernel build
# ---------------------------------------------------------------------------


def _build():
    nc = bass.Bass("TRN2", target_bir_lowering=False, debug=False, num_devices=NCORES)
    # x pre-transposed on host: [X, L*BC] bf16 (row p = x[:, :, p] flattened)
    xd = nc.dram_tensor("xT", [X, L * BC], BF16, kind="ExternalInput").ap()
    wihT = nc.dram_tensor("wihT", [X, G4], BF16, kind="ExternalInput").ap()
    whhT = nc.dram_tensor("whhT", [H, G4], BF16, kind="ExternalInput").ap()
    biasd = nc.dram_tensor("bias", [1, G4], BF16, kind="ExternalInput").ap()
    biasf_d = nc.dram_tensor("biasf", [1, G4], F32, kind="ExternalInput").ap()
    w0T = nc.dram_tensor("w0T", [H, H], BF16, kind="ExternalInput").ap()
    b0d = nc.dram_tensor("b0", [1, H], F32, kind="ExternalInput").ap()
    w1T = nc.dram_tensor("w1T", [H, H], BF16, kind="ExternalInput").ap()
    b1d = nc.dram_tensor("b1", [1, H], F32, kind="ExternalInput").ap()
    w2T = nc.dram_tensor("w2T", [H, 1], BF16, kind="ExternalInput").ap()
    b2d = nc.dram_tensor("b2", [1, 1], F32, kind="ExternalInput").ap()
    outd = nc.dram_tensor("out", [BC, 1], F32, kind="ExternalOutput").ap()

    with tile.TileContext(nc) as tc:
        with (
            tc.tile_pool(name="consts", bufs=1) as cp,
            tc.tile_pool(name="gact", bufs=2) as gp,
            tc.tile_pool(name="small", bufs=2) as sp,
            tc.tile_pool(name="gps", bufs=2, space="PSUM") as pg,
            tc.tile_pool(name="tps", bufs=1, space="PSUM") as pt,
            tc.tile_pool(name="jps", bufs=1, space="PSUM") as pj,
            tc.tile_pool(name="mps", bufs=1, space="PSUM") as pm,
        ):
            # ---- constants ----
            ident = cp.tile([128, 128], F32)
            make_identity(nc, ident)
            ones_sb = cp.tile([1, 128], BF16)
            nc.vector.memset(ones_sb, 1.0)
            wih_sb = cp.tile([128, G4], BF16)
            nc.sync.dma_start(out=wih_sb, in_=wihT)
            whh_sb = cp.tile([128, 2, G4], BF16)
            nc.sync.dma_start(
                out=whh_sb, in_=whhT.rearrange("(k p) n -> p k n", p=128)
            )
            biasr_sb = cp.tile([1, G4], BF16)
            nc.sync.dma_start(out=biasr_sb, in_=biasd)
            w0_sb = cp.tile([128, 2, H], BF16)
            nc.sync.dma_start(out=w0_sb, in_=w0T.rearrange("(k p) n -> p k n", p=128))
            b0_sb = cp.tile([128, H], F32)
            nc.sync.dma_start(out=b0_sb, in_=_bcast(b0d, 128))
            w1_sb = cp.tile([128, 2, H], BF16)
            nc.sync.dma_start(out=w1_sb, in_=w1T.rearrange("(k p) n -> p k n", p=128))
            b1_sb = cp.tile([128, H], F32)
            nc.sync.dma_start(out=b1_sb, in_=_bcast(b1d, 128))
            w2_sb = cp.tile([128, 2, 1], BF16)
            nc.sync.dma_start(out=w2_sb, in_=w2T.rearrange("(k p) n -> p k n", p=128))
            b2_sb = cp.tile([128, 1], F32)
            nc.sync.dma_start(out=b2_sb, in_=_bcast(b2d, 128))

            # ---- whole-x preload: XCHUNK-step tiles ----
            nxt = L // XCHUNK
            x_sb = []
            for k in range(nxt):
                xt = cp.tile([128, XCHUNK, BC], BF16)
                nc.sync.dma_start(
                    out=xt,
                    in_=xd[:, k * XCHUNK * BC : (k + 1) * XCHUNK * BC].rearrange(
                        "p (t b) -> p t b", t=XCHUNK
                    ),
                )
                x_sb.append(xt)

            # ---- initial state ----
            c_cur = sp.tile([128, H], BF16, tag="c")
            nc.vector.memset(c_cur, 0.0)
            hT_cur = sp.tile([128, 2, BC], BF16, tag="hT")
            nc.vector.memset(hT_cur, 0.0)
            identb = cp.tile([128, 128], BF16)
            nc.vector.tensor_copy(identb, ident)

            # fp32 broadcast bias tiles for the DVE PSUM preload
            b0cast = cp.tile([128, 512], F32)
            nc.sync.dma_start(out=b0cast, in_=_bcast(biasf_d[:, 0:512], 128))
            b1cast = cp.tile([128, 512], F32)
            nc.sync.dma_start(out=b1cast, in_=_bcast(biasf_d[:, 512:1024], 128))

            def emit_bias_x_mm(G0, G1, t):
                # warmup path (t < 2): bias via K=1 ones matmul, start=True
                # sets every PSUM has_written bit in the bank
                xtT = x_sb[t // XCHUNK][:, t % XCHUNK, :]
                nc.tensor.matmul(G0, ones_sb, biasr_sb[:, S0], start=True, stop=False)
                nc.tensor.matmul(G1, ones_sb, biasr_sb[:, S1], start=True, stop=False)
                nc.tensor.matmul(G0, xtT, wih_sb[:, S0], start=False, stop=False)
                nc.tensor.matmul(G1, xtT, wih_sb[:, S1], start=False, stop=False)

            def emit_bias_x_dve(G0, G1, t, t12):
                # steady state (t >= 2): every has_written bit in these banks
                # is already set (start=True two steps ago, never cleared
                # since), so DVE/ACT write the bias values and all matmuls
                # accumulate with start=False. The DVE write is expressed as
                # 0*t12 + bias so it cannot jump ahead of the chain ops in
                # the out-of-order engine queue.
                xtT = x_sb[t // XCHUNK][:, t % XCHUNK, :]
                nc.vector.scalar_tensor_tensor(G0, t12, 0.0, b0cast,
                                               ALU.mult, ALU.add)
                nc.scalar.copy(G1, b1cast)
                nc.tensor.matmul(G0, xtT, wih_sb[:, S0], start=False, stop=False,
                                 skip_group_check=True)
                nc.tensor.matmul(G1, xtT, wih_sb[:, S1], start=False, stop=False,
                                 skip_group_check=True)

            # ---- recurrence ----
            # G split into two PSUM tiles so activations wait only on their
            # own slice's matmuls (precise semaphore targets)
            G0_cur = pg.tile([128, 512], F32, tag="G0")  # gates i, f
            G1_cur = pg.tile([128, 512], F32, tag="G1")  # gates o, g
            emit_bias_x_mm(G0_cur, G1_cur, 0)
            for t in range(L):
                G0, G1 = G0_cur, G1_cur
                # h matmuls: G0 (i,f) completes first so sigmoid starts
                # earliest; G1 leads with the h1 chunk so the engine cannot
                # commit h0-G1 into the slot h1-G0 needs
                nc.tensor.matmul(G0, hT_cur[:, 0, :], whh_sb[:, 0, S0],
                                 start=False, stop=False, skip_group_check=True)
                nc.tensor.matmul(G0, hT_cur[:, 1, :], whh_sb[:, 1, S0],
                                 start=False, stop=True, skip_group_check=True)
                nc.tensor.matmul(G1, hT_cur[:, 1, :], whh_sb[:, 1, S1],
                                 start=False, stop=False, skip_group_check=True)
                nc.tensor.matmul(G1, hT_cur[:, 0, :], whh_sb[:, 0, S1],
                                 start=False, stop=True, skip_group_check=True)

                # activations straight from PSUM, bf16 outputs
                sif = gp.tile([128, 512], BF16, tag="sif")
                nc.scalar.activation(sif, G0, AF.Sigmoid)
                tg = gp.tile([128, H], BF16, tag="tg")
                nc.scalar.activation(tg, G1[:, 256:512], AF.Tanh)
                o_sb = gp.tile([128, H], BF16, tag="o")
                nc.scalar.activation(o_sb, G1[:, 0:256], AF.Sigmoid)

                # c update, all on DVE in bf16 (t1 first: only needs sig(f));
                # t1/t2 share one tile so the next bias preload can depend on
                # both via a single read
                t12 = sp.tile([128, 512], BF16, tag="t12")
                nc.vector.tensor_mul(t12[:, 0:256], sif[:, 256:512], c_cur)
                nc.vector.tensor_mul(t12[:, 256:512], sif[:, 0:256], tg)
                c_new = sp.tile([128, H], BF16, tag="c")
                nc.vector.tensor_add(c_new, t12[:, 0:256], t12[:, 256:512])

                # mult-last tail: transpose o (off-path) and c, tanh on the
                # transposed c, multiply straight into SBUF hT (no copy)
                oT_ps = pt.tile([128, 2, BC], BF16, tag="oTps")
                nc.tensor.transpose(oT_ps[:, 0, :], o_sb[:, 0:128], identb)
                nc.tensor.transpose(oT_ps[:, 1, :], o_sb[:, 128:256], identb)
                cT_ps = pt.tile([128, 2, BC], BF16, tag="cTps")
                nc.tensor.transpose(cT_ps[:, 0, :], c_new[:, 0:128], identb)
                nc.tensor.transpose(cT_ps[:, 1, :], c_new[:, 128:256], identb)
                tancT = sp.tile([128, 2, BC], BF16, tag="tancT")
                hT_new = sp.tile([128, 2, BC], BF16, tag="hT")
                nc.scalar.activation(tancT[:, 0, :], cT_ps[:, 0, :], AF.Tanh)
                nc.vector.tensor_mul(hT_new[:, 0, :], oT_ps[:, 0, :], tancT[:, 0, :])
                nc.scalar.activation(tancT[:, 1, :], cT_ps[:, 1, :], AF.Tanh)
                nc.vector.tensor_mul(hT_new[:, 1, :], oT_ps[:, 1, :], tancT[:, 1, :])
                # next step's bias preload (DVE) + x matmuls, emitted last so
                # the DVE copies queue behind this step's chain ops and run
                # in DVE's idle window
                if t + 1 < L:
                    G0_cur = pg.tile([128, 512], F32, tag="G0")
                    G1_cur = pg.tile([128, 512], F32, tag="G1")
                    if t + 1 < 2:
                        emit_bias_x_mm(G0_cur, G1_cur, t + 1)
                    else:
                        emit_bias_x_dve(G0_cur, G1_cur, t + 1, t12)
                hT_cur = hT_new
                c_cur = c_new

            # ---- MLP head: leaky_relu(z) = max(z,0) + 0.2*min(z,0) ----
            def linear(hT0, hT1, w_sb, b_sb, n_out):
                m = pm.tile([128, n_out], F32, tag="mlp_ps")
                nc.tensor.matmul(m, hT0, w_sb[:, 0, :], start=True, stop=False)
                nc.tensor.matmul(m, hT1, w_sb[:, 1, :], start=False, stop=True)
                z = sp.tile([128, n_out], F32, tag="mlp_z")
                nc.vector.tensor_add(z, m, b_sb[:, 0:n_out])
                return z

            def leaky(z, n_out):
                neg = sp.tile([128, n_out], F32, tag="mlp_neg")
                nc.vector.tensor_scalar(neg, z, 0.0, 0.2, ALU.min, ALU.mult)
                a = sp.tile([128, n_out], F32, tag="mlp_a")
                nc.vector.scalar_tensor_tensor(a, z, 0.0, neg, ALU.max, ALU.add)
                return a

            def transpose_act(a):
                aT_ps = pj.tile([128, 512], F32, tag="junk")
                nc.tensor.transpose(aT_ps[:, 0:128], a[:, 0:128], ident)
                nc.tensor.transpose(aT_ps[:, 128:256], a[:, 128:256], ident)
                aT = sp.tile([128, H], BF16, tag="mlp_aTsb")
                nc.vector.tensor_copy(aT, aT_ps[:, 0:256])
                return aT[:, 0:128], aT[:, 128:256]

            z0 = linear(hT_cur[:, 0, :], hT_cur[:, 1, :], w0_sb, b0_sb, H)
            a0 = leaky(z0, H)
            a0T0, a0T1 = transpose_act(a0)
            z1 = linear(a0T0, a0T1, w1_sb, b1_sb, H)
            a1 = leaky(z1, H)
            a1T0, a1T1 = transpose_act(a1)
            z2 = linear(a1T0, a1T1, w2_sb, b2_sb, 1)
            nc.sync.dma_start(out=outd, in_=z2)

    return nc


# ---------------------------------------------------------------------------
# Host-side driver with cached compiled executable
# ---------------------------------------------------------------------------

_CACHE = {}


def _get_exec():
    if "exec" in _CACHE:
        return _CACHE["exec"]
    _install_fixup()
    bass2jax.install_neuronx_cc_hook()
    import jax

    nc = _build()

    part_name = nc.partition_id_tensor.name if nc.partition_id_tensor else None
    in_names, out_names, out_avals, zero_shapes = [], [], [], []
    for alloc in nc.m.functions[0].allocations:
        if not isinstance(alloc, mybir.MemoryLocationSet):
            continue
        name = alloc.memorylocations[0].name
        if alloc.kind == "ExternalInput":
            if name != part_name:
                in_names.append(name)
        elif alloc.kind == "ExternalOutput":
            out_names.append(name)
            shape = tuple(alloc.tensor_shape)
            dtype = mybir.dt.np(alloc.dtype)
            out_avals.append(jax.core.ShapedArray(shape, dtype))
            zero_shapes.append((shape, dtype))
    n_params = len(in_names)
    n_outs = len(out_names)
    all_in_names = in_names + out_names
    if part_name is not None:
        all_in_names = all_in_names + [part_name]
    donate = tuple(range(n_params, n_params + n_outs))

    def _body(*args):
        operands = list(args)
        if part_name is not None:
            operands.append(bass2jax.partition_id_tensor())
        outs = bass2jax._bass_exec_p.bind(
            *operands,
            out_avals=tuple(out_avals),
            in_names=tuple(all_in_names),
            out_names=tuple(out_names),
            lowering_input_output_aliases=(),
            sim_require_finite=True,
            sim_require_nnan=True,
            nc=nc,
        )
        return tuple(outs)

    devices = jax.devices()[:NCORES]
    mesh = bass2jax.Mesh(np.asarray(devices), ("core",))
    spec = (bass2jax.PartitionSpec("core"),)
    sharded = jax.jit(
        bass2jax.shard_map(
            _body,
            mesh=mesh,
            in_specs=spec * (n_params + n_outs),
            out_specs=spec * n_outs,
            check_rep=False,
        ),
        donate_argnums=donate,
        keep_unused=True,
    )
    _CACHE["exec"] = (sharded, in_names, out_names, zero_shapes)
    return _CACHE["exec"]


def _prep_inputs(x, W_ih, W_hh, b_ih, b_hh, W0, b0, W1, b1, W2, b2):
    import ml_dtypes

    bf = ml_dtypes.bfloat16
    # gate reorder (i,f,g,o) -> (i,f,o,g)
    idx = np.concatenate(
        [
            np.arange(0, 256),      # i
            np.arange(256, 512),    # f
            np.arange(768, 1024),   # o
            np.arange(512, 768),    # g
        ]
    )
    wih_p = W_ih[idx].astype(np.float32)
    whh_p = W_hh[idx].astype(np.float32)
    bias_p = (b_ih + b_hh)[idx].astype(np.float32)

    per_core_common = {
        "wihT": np.ascontiguousarray(wih_p.T).astype(bf),
        "whhT": np.ascontiguousarray(whh_p.T).astype(bf),
        "bias": bias_p.reshape(1, G4).astype(bf),
        "biasf": bias_p.reshape(1, G4).astype(np.float32),
        "w0T": np.ascontiguousarray(W0.T).astype(bf),
        "b0": b0.reshape(1, H).astype(np.float32),
        "w1T": np.ascontiguousarray(W1.T).astype(bf),
        "b1": b1.reshape(1, H).astype(np.float32),
        "w2T": np.ascontiguousarray(W2.T).astype(bf),
        "b2": b2.reshape(1, 1).astype(np.float32),
    }
    xbf = np.asarray(x).astype(bf)  # [L, B, X]
    in_maps = []
    for i in range(NCORES):
        m = dict(per_core_common)
        xc = xbf[:, i * BC : (i + 1) * BC, :]  # [L, BC, X]
        m["xT"] = np.ascontiguousarray(xc.transpose(2, 0, 1)).reshape(X, L * BC)
        in_maps.append(m)
    return in_maps


def _concat_inputs(in_maps, in_names):
    return [
        np.concatenate([np.asarray(in_maps[c][n]) for c in range(NCORES)], axis=0)
        for n in in_names
    ]


def _run_concat(concat_in):
    sharded, in_names, out_names, zero_shapes = _get_exec()
    zeros = [np.zeros((NCORES * s[0],) + s[1:], d) for s, d in zero_shapes]
    out_arrs = sharded(*concat_in, *zeros)
    return np.asarray(out_arrs[0])  # [8*BC, 1]


def kernel(**inputs) -> np.ndarray:
    sharded, in_names, out_names, zero_shapes = _get_exec()
    in_maps = _prep_inputs(**{k: np.asarray(v) for k, v in inputs.items()})
    concat_in = _concat_inputs(in_maps, in_names)
    out = _run_concat(concat_in)
    return out.reshape(B, 1).astype(np.float32)


def timed_run(inputs, iters=5):
    """Returns (best_seconds, output). Inputs transferred to device once."""
    import jax

    sharded, in_names, out_names, zero_shapes = _get_exec()
    in_maps = _prep_inputs(**{k: np.asarray(v) for k, v in inputs.items()})
    concat_in = _concat_inputs(in_maps, in_names)
    out = _run_concat(concat_in)  # compile + warm
    mesh = bass2jax.Mesh(np.asarray(jax.devices()[:NCORES]), ("core",))
    shd = jax.sharding.NamedSharding(mesh, bass2jax.PartitionSpec("core"))
    dev_in = [jax.device_put(a, shd) for a in concat_in]
    times = []
    for _ in range(iters):
        zeros = [np.zeros((NCORES * s[0],) + s[1:], d) for s, d in zero_shapes]
        t0 = time.perf_counter()
        r = sharded(*dev_in, *zeros)
        jax.block_until_ready(r)
        times.append(time.perf_counter() - t0)
    return min(times), out.reshape(B, 1)


# revision 26
# speedup vs baseline: 24.4672x; 1.0009x over previous
"""Trainium2 Bass kernel for nn_LSTMDiscriminator.

LSTM (L=512, B=1024, X=128, H=256) + 3-layer MLP head, data-parallel over
batch across 8 NeuronCores (128 samples per core).

v2 design (per core, batch-on-partitions layout G[B=128, 4H]):
  - all matmul operands bf16 (fp32 PSUM accumulate): 4x PE throughput vs fp32
  - x pre-transposed on host to [X, L*BC] and preloaded whole into SBUF
    (128 KiB/partition) -> zero in-loop DMA
  - gate bias enters PSUM via a K=1 ones-matmul, so activations read PSUM
    directly and DVE does no bias adds
  - gate order (i, f | o, g): sigmoid(i,f) after slice0 stops, tanh(g)
    right after slice1 stops (critical path), sigmoid(o) off-path
  - tail: transpose c_new on PE, tanh on the transposed tile, then
    hT = tanh(cT) * oT in one DVE op writing bf16 hT straight to SBUF
    (h is only ever carried transposed - no h transpose / copy)
"""

import sys
import time

sys.path.insert(0, "/opt/trn_rl_repo")

import json
import numpy as np

import concourse.bass as bass
import concourse.tile as tile
from concourse import mybir
from concourse import bass2jax
from concourse.masks import make_identity

L, B, X, H = 512, 1024, 128, 256
NCORES = 8
BC = B // NCORES  # 128 per core
G4 = 4 * H  # 1024
F32 = mybir.dt.float32
BF16 = mybir.dt.bfloat16
AF = mybir.ActivationFunctionType
ALU = mybir.AluOpType

XCHUNK = 64  # steps per x-preload DMA
S0 = slice(0, 512)      # gates i (0:256), f (256:512)
S1 = slice(512, 1024)   # gates o (512:768), g (768:1024)

# ---------------------------------------------------------------------------
# Workaround: this walrus build accepts only ONE sync-wait per instruction.
# Split any instruction with N>1 on_wait conditions into N-1 single-wait
# NoOp carriers (same engine, program order preserved) + the instruction.
# ---------------------------------------------------------------------------


def _split_multi_waits(bir: dict) -> int:
    n_split = 0
    for fn in bir.get("functions", []):
        for blk in fn.get("blocks", []):
            out = []
            for inst in blk.get("instructions", []):
                si = inst.get("sync_info")
                waits = (si or {}).get("on_wait") or []
                if len(waits) > 1:
                    for k, w in enumerate(waits[:-1]):
                        out.append(
                            {
                                "debug": inst.get("debug", 0),
                                "engine": inst.get("engine"),
                                "ins": [],
                                "name": f"{inst['name']}-ws{k}",
                                "opcode": "NoOp",
                                "outs": [],
                                "sync_info": {"on_update": [], "on_wait": [w]},
                            }
                        )
                    si["on_wait"] = [waits[-1]]
                    n_split += 1
                out.append(inst)
            blk["instructions"] = out
    return n_split


def _install_fixup():
    from concourse import bass_utils

    if getattr(bass_utils, "_lstm_fixup_installed", False):
        return
    orig = bass_utils.compile_bir_kernel

    def wrapper(ant_bir_str, compile_dir_path, neff_name="file.neff", **kw):
        bir = json.loads(ant_bir_str)
        _split_multi_waits(bir)
        return orig(json.dumps(bir).encode(), compile_dir_path, neff_name=neff_name, **kw)

    bass_utils.compile_bir_kernel = wrapper
    bass_utils._lstm_fixup_installed = True
    bass2jax.compile_bir_kernel = wrapper


def _bcast(ap, n):
    """View a [1, m] DRAM AP as [n, m] via zero partition stride."""
    return bass.AP(tensor=ap.tensor, offset=ap.offset, ap=[[0, n]] + list(ap.ap[1:]))


# ---------------------------------------------------------------------------
# Kernel build
# ---------------------------------------------------------------------------


def _build():
    nc = bass.Bass("TRN2", target_bir_lowering=False, debug=False, num_devices=NCORES)
    # x pre-transposed on host: [X, L*BC] bf16 (row p = x[:, :, p] flattened)
    xd = nc.dram_tensor("xT", [X, L * BC], BF16, kind="ExternalInput").ap()
    wihT = nc.dram_tensor("wihT", [X, G4], BF16, kind="ExternalInput").ap()
    whhT = nc.dram_tensor("whhT", [H, G4], BF16, kind="ExternalInput").ap()
    biasd = nc.dram_tensor("bias", [1, G4], BF16, kind="ExternalInput").ap()
    biasf_d = nc.dram_tensor("biasf", [1, G4], F32, kind="ExternalInput").ap()
    w0T = nc.dram_tensor("w0T", [H, H], BF16, kind="ExternalInput").ap()
    b0d = nc.dram_tensor("b0", [1, H], F32, kind="ExternalInput").ap()
    w1T = nc.dram_tensor("w1T", [H, H], BF16, kind="ExternalInput").ap()
    b1d = nc.dram_tensor("b1", [1, H], F32, kind="ExternalInput").ap()
    w2T = nc.dram_tensor("w2T", [H, 1], BF16, kind="ExternalInput").ap()
    b2d = nc.dram_tensor("b2", [1, 1], F32, kind="ExternalInput").ap()
    outd = nc.dram_tensor("out", [BC, 1], F32, kind="ExternalOutput").ap()

    with tile.TileContext(nc) as tc:
        with (
            tc.tile_pool(name="consts", bufs=1) as cp,
            tc.tile_pool(name="gact", bufs=2) as gp,
            tc.tile_pool(name="small", bufs=2) as sp,
            tc.tile_pool(name="gps", bufs=2, space="PSUM") as pg,
            tc.tile_pool(name="tps", bufs=1, space="PSUM") as pt,
            tc.tile_pool(name="jps", bufs=1, space="PSUM") as pj,
            tc.tile_pool(name="mps", bufs=1, space="PSUM") as pm,
        ):
            # ---- constants ----
            ident = cp.tile([128, 128], F32)
            make_identity(nc, ident)
            ones_sb = cp.tile([1, 128], BF16)
            nc.vector.memset(ones_sb, 1.0)
            wih_sb = cp.tile([128, G4], BF16)
            nc.sync.dma_start(out=wih_sb, in_=wihT)
            whh_sb = cp.tile([128, 2, G4], BF16)
            nc.sync.dma_start(
                out=whh_sb, in_=whhT.rearrange("(k p) n -> p k n", p=128)
            )
            biasr_sb = cp.tile([1, G4], BF16)
            nc.sync.dma_start(out=biasr_sb, in_=biasd)
            w0_sb = cp.tile([128, 2, H], BF16)
            nc.sync.dma_start(out=w0_sb, in_=w0T.rearrange("(k p) n -> p k n", p=128))
            b0_sb = cp.tile([128, H], F32)
            nc.sync.dma_start(out=b0_sb, in_=_bcast(b0d, 128))
            w1_sb = cp.tile([128, 2, H], BF16)
            nc.sync.dma_start(out=w1_sb, in_=w1T.rearrange("(k p) n -> p k n", p=128))
            b1_sb = cp.tile([128, H], F32)
            nc.sync.dma_start(out=b1_sb, in_=_bcast(b1d, 128))
            w2_sb = cp.tile([128, 2, 1], BF16)
            nc.sync.dma_start(out=w2_sb, in_=w2T.rearrange("(k p) n -> p k n", p=128))
            b2_sb = cp.tile([128, 1], F32)
            nc.sync.dma_start(out=b2_sb, in_=_bcast(b2d, 128))

            # ---- whole-x preload: XCHUNK-step tiles ----
            nxt = L // XCHUNK
            x_sb = []
            for k in range(nxt):
                xt = cp.tile([128, XCHUNK, BC], BF16)
                nc.sync.dma_start(
                    out=xt,
                    in_=xd[:, k * XCHUNK * BC : (k + 1) * XCHUNK * BC].rearrange(
                        "p (t b) -> p t b", t=XCHUNK
                    ),
                )
                x_sb.append(xt)

            # ---- initial state ----
            c_cur = sp.tile([128, H], BF16, tag="c")
            nc.vector.memset(c_cur, 0.0)
            hT_cur = sp.tile([128, 2, BC], BF16, tag="hT")
            nc.vector.memset(hT_cur, 0.0)
            identb = cp.tile([128, 128], BF16)
            nc.vector.tensor_copy(identb, ident)

            # fp32 broadcast bias tiles for the DVE PSUM preload
            b0cast = cp.tile([128, 512], F32)
            nc.sync.dma_start(out=b0cast, in_=_bcast(biasf_d[:, 0:512], 128))
            b1cast = cp.tile([128, 512], F32)
            nc.sync.dma_start(out=b1cast, in_=_bcast(biasf_d[:, 512:1024], 128))

            def emit_bias_x_mm(G0, G1, t):
                # warmup path (t < 2): bias via K=1 ones matmul, start=True
                # sets every PSUM has_written bit in the bank
                xtT = x_sb[t // XCHUNK][:, t % XCHUNK, :]
                nc.tensor.matmul(G0, ones_sb, biasr_sb[:, S0], start=True, stop=False)
                nc.tensor.matmul(G1, ones_sb, biasr_sb[:, S1], start=True, stop=False)
                nc.tensor.matmul(G0, xtT, wih_sb[:, S0], start=False, stop=False)
                nc.tensor.matmul(G1, xtT, wih_sb[:, S1], start=False, stop=False)

            def emit_bias_x_dve(G0, G1, t, t12):
                # steady state (t >= 2): every has_written bit in these banks
                # is already set (start=True two steps ago, never cleared
                # since), so DVE/ACT write the bias values and all matmuls
                # accumulate with start=False. The DVE write is expressed as
                # 0*t12 + bias so it cannot jump ahead of the chain ops in
                # the out-of-order engine queue.
                xtT = x_sb[t // XCHUNK][:, t % XCHUNK, :]
                nc.vector.scalar_tensor_tensor(G0, t12, 0.0, b0cast,
                                               ALU.mult, ALU.add)
                nc.scalar.copy(G1, b1cast)
                nc.tensor.matmul(G0, xtT, wih_sb[:, S0], start=False, stop=False,
                                 skip_group_check=True)
                nc.tensor.matmul(G1, xtT, wih_sb[:, S1], start=False, stop=False,
                                 skip_group_check=True)

            # ---- recurrence ----
            # G split into two PSUM tiles so activations wait only on their
            # own slice's matmuls (precise semaphore targets)
            G0_cur = pg.tile([128, 512], F32, tag="G0")  # gates i, f
            G1_cur = pg.tile([128, 512], F32, tag="G1")  # gates o, g
            emit_bias_x_mm(G0_cur, G1_cur, 0)
            for t in range(L):
                G0, G1 = G0_cur, G1_cur
                # h matmuls: G0 (i,f) completes first so sigmoid starts
                # earliest; G1 leads with the h1 chunk so the engine cannot
                # commit h0-G1 into the slot h1-G0 needs
                nc.tensor.matmul(G0, hT_cur[:, 0, :], whh_sb[:, 0, S0],
                                 start=False, stop=False, skip_group_check=True)
                nc.tensor.matmul(G0, hT_cur[:, 1, :], whh_sb[:, 1, S0],
                                 start=False, stop=True, skip_group_check=True)
                nc.tensor.matmul(G1, hT_cur[:, 1, :], whh_sb[:, 1, S1],
                                 start=False, stop=False, skip_group_check=True)
                nc.tensor.matmul(G1, hT_cur[:, 0, :], whh_sb[:, 0, S1],
                                 start=False, stop=True, skip_group_check=True)

                # activations straight from PSUM, bf16 outputs
                sif = gp.tile([128, 512], BF16, tag="sif")
                nc.scalar.activation(sif, G0, AF.Sigmoid)
                tg = gp.tile([128, H], BF16, tag="tg")
                nc.scalar.activation(tg, G1[:, 256:512], AF.Tanh)
                o_sb = gp.tile([128, H], BF16, tag="o")
                nc.scalar.activation(o_sb, G1[:, 0:256], AF.Sigmoid)

                # c update, all on DVE in bf16 (t1 first: only needs sig(f));
                # t1/t2 share one tile so the next bias preload can depend on
                # both via a single read
                t12 = sp.tile([128, 512], BF16, tag="t12")
                nc.vector.tensor_mul(t12[:, 0:256], sif[:, 256:512], c_cur)
                nc.vector.tensor_mul(t12[:, 256:512], sif[:, 0:256], tg)
                c_new = sp.tile([128, H], BF16, tag="c")
                nc.vector.tensor_add(c_new, t12[:, 0:256], t12[:, 256:512])

                # mult-last tail: transpose o (off-path) and c, tanh on the
                # transposed c, multiply straight into SBUF hT (no copy)
                oT_ps = pt.tile([128, 2, BC], BF16, tag="oTps")
                nc.tensor.transpose(oT_ps[:, 0, :], o_sb[:, 0:128], identb)
                nc.tensor.transpose(oT_ps[:, 1, :], o_sb[:, 128:256], identb)
                cT_ps = pt.tile([128, 2, BC], BF16, tag="cTps")
                nc.tensor.transpose(cT_ps[:, 0, :], c_new[:, 0:128], identb)
                nc.tensor.transpose(cT_ps[:, 1, :], c_new[:, 128:256], identb)
                tancT = sp.tile([128, 2, BC], BF16, tag="tancT")
                hT_new = sp.tile([128, 2, BC], BF16, tag="hT")
                nc.scalar.activation(tancT[:, 0, :], cT_ps[:, 0, :], AF.Tanh)
                nc.vector.tensor_mul(hT_new[:, 0, :], oT_ps[:, 0, :], tancT[:, 0, :])
                nc.scalar.activation(tancT[:, 1, :], cT_ps[:, 1, :], AF.Tanh)
                nc.vector.tensor_mul(hT_new[:, 1, :], oT_ps[:, 1, :], tancT[:, 1, :])
                # next step's bias preload (DVE) + x matmuls, emitted last so
                # the DVE copies queue behind this step's chain ops and run
                # in DVE's idle window
                if t + 1 < L:
                    G0_cur = pg.tile([128, 512], F32, tag="G0")
                    G1_cur = pg.tile([128, 512], F32, tag="G1")
                    if t + 1 < 2:
                        emit_bias_x_mm(G0_cur, G1_cur, t + 1)
                    else:
                        emit_bias_x_dve(G0_cur, G1_cur, t + 1, t12)
                hT_cur = hT_new
                c_cur = c_new

            # ---- MLP head: leaky_relu(z) = max(z,0) + 0.2*min(z,0) ----
            def linear(hT0, hT1, w_sb, b_sb, n_out):
                m = pm.tile([128, n_out], F32, tag="mlp_ps")
                nc.tensor.matmul(m, hT0, w_sb[:, 0, :], start=True, stop=False)
                nc.tensor.matmul(m, hT1, w_sb[:, 1, :], start=False, stop=True)
                z = sp.tile([128, n_out], F32, tag="mlp_z")
                nc.vector.tensor_add(z, m, b_sb[:, 0:n_out])
                return z

            def leaky(z, n_out):
                neg = sp.tile([128, n_out], F32, tag="mlp_neg")
                nc.vector.tensor_scalar(neg, z, 0.0, 0.2, ALU.min, ALU.mult)
                a = sp.tile([128, n_out], F32, tag="mlp_a")
                nc.vector.scalar_tensor_tensor(a, z, 0.0, neg, ALU.max, ALU.add)
                return a

            def transpose_act(a):
                aT_ps = pj.tile([128, 512], F32, tag="junk")
                nc.tensor.transpose(aT_ps[:, 0:128], a[:, 0:128], ident)
                nc.tensor.transpose(aT_ps[:, 128:256], a[:, 128:256], ident)
                aT = sp.tile([128, H], BF16, tag="mlp_aTsb")
                nc.vector.tensor_copy(aT, aT_ps[:, 0:256])
                return aT[:, 0:128], aT[:, 128:256]

            z0 = linear(hT_cur[:, 0, :], hT_cur[:, 1, :], w0_sb, b0_sb, H)
            a0 = leaky(z0, H)
            a0T0, a0T1 = transpose_act(a0)
            z1 = linear(a0T0, a0T1, w1_sb, b1_sb, H)
            a1 = leaky(z1, H)
            a1T0, a1T1 = transpose_act(a1)
            z2 = linear(a1T0, a1T1, w2_sb, b2_sb, 1)
            nc.sync.dma_start(out=outd, in_=z2)

    return nc


# ---------------------------------------------------------------------------
# Host-side driver with cached compiled executable
# ---------------------------------------------------------------------------

_CACHE = {}


def _get_exec():
    if "exec" in _CACHE:
        return _CACHE["exec"]
    _install_fixup()
    bass2jax.install_neuronx_cc_hook()
    import jax

    nc = _build()

    part_name = nc.partition_id_tensor.name if nc.partition_id_tensor else None
    in_names, out_names, out_avals, zero_shapes = [], [], [], []
    for alloc in nc.m.functions[0].allocations:
        if not isinstance(alloc, mybir.MemoryLocationSet):
            continue
        name = alloc.memorylocations[0].name
        if alloc.kind == "ExternalInput":
            if name != part_name:
                in_names.append(name)
        elif alloc.kind == "ExternalOutput":
            out_names.append(name)
            shape = tuple(alloc.tensor_shape)
            dtype = mybir.dt.np(alloc.dtype)
            out_avals.append(jax.core.ShapedArray(shape, dtype))
            zero_shapes.append((shape, dtype))
    n_params = len(in_names)
    n_outs = len(out_names)
    all_in_names = in_names + out_names
    if part_name is not None:
        all_in_names = all_in_names + [part_name]
    donate = tuple(range(n_params, n_params + n_outs))

    def _body(*args):
        operands = list(args)
        if part_name is not None:
            operands.append(bass2jax.partition_id_tensor())
        outs = bass2jax._bass_exec_p.bind(
            *operands,
            out_avals=tuple(out_avals),
            in_names=tuple(all_in_names),
            out_names=tuple(out_names),
            lowering_input_output_aliases=(),
            sim_require_finite=True,
            sim_require_nnan=True,
            nc=nc,
        )
        return tuple(outs)

    devices = jax.devices()[:NCORES]
    mesh = bass2jax.Mesh(np.asarray(devices), ("core",))
    spec = (bass2jax.PartitionSpec("core"),)
    sharded = jax.jit(
        bass2jax.shard_map(
            _body,
            mesh=mesh,
            in_specs=spec * (n_params + n_outs),
            out_specs=spec * n_outs,
            check_rep=False,
        ),
        donate_argnums=donate,
        keep_unused=True,
    )
    _CACHE["exec"] = (sharded, in_names, out_names, zero_shapes)
    return _CACHE["exec"]


def _prep_inputs(x, W_ih, W_hh, b_ih, b_hh, W0, b0, W1, b1, W2, b2):
    import ml_dtypes

    bf = ml_dtypes.bfloat16
    # gate reorder (i,f,g,o) -> (i,f,o,g)
    idx = np.concatenate(
        [
            np.arange(0, 256),      # i
            np.arange(256, 512),    # f
            np.arange(768, 1024),   # o
            np.arange(512, 768),    # g
        ]
    )
    wih_p = W_ih[idx].astype(np.float32)
    whh_p = W_hh[idx].astype(np.float32)
    bias_p = (b_ih + b_hh)[idx].astype(np.float32)

    per_core_common = {
        "wihT": np.ascontiguousarray(wih_p.T).astype(bf),
        "whhT": np.ascontiguousarray(whh_p.T).astype(bf),
        "bias": bias_p.reshape(1, G4).astype(bf),
        "biasf": bias_p.reshape(1, G4).astype(np.float32),
        "w0T": np.ascontiguousarray(W0.T).astype(bf),
        "b0": b0.reshape(1, H).astype(np.float32),
        "w1T": np.ascontiguousarray(W1.T).astype(bf),
        "b1": b1.reshape(1, H).astype(np.float32),
        "w2T": np.ascontiguousarray(W2.T).astype(bf),
        "b2": b2.reshape(1, 1).astype(np.float32),
    }
    xbf = np.asarray(x).astype(bf)  # [L, B, X]
    in_maps = []
    for i in range(NCORES):
        m = dict(per_core_common)
        xc = xbf[:, i * BC : (i + 1) * BC, :]  # [L, BC, X]
        m["xT"] = np.ascontiguousarray(xc.transpose(2, 0, 1)).reshape(X, L * BC)
        in_maps.append(m)
    return in_maps


def _concat_inputs(in_maps, in_names):
    return [
        np.concatenate([np.asarray(in_maps[c][n]) for c in range(NCORES)], axis=0)
        for n in in_names
    ]


def _run_concat(concat_in):
    sharded, in_names, out_names, zero_shapes = _get_exec()
    zeros = [np.zeros((NCORES * s[0],) + s[1:], d) for s, d in zero_shapes]
    out_arrs = sharded(*concat_in, *zeros)
    return np.asarray(out_arrs[0])  # [8*BC, 1]


def kernel(**inputs) -> np.ndarray:
    sharded, in_names, out_names, zero_shapes = _get_exec()
    in_maps = _prep_inputs(**{k: np.asarray(v) for k, v in inputs.items()})
    concat_in = _concat_inputs(in_maps, in_names)
    out = _run_concat(concat_in)
    return out.reshape(B, 1).astype(np.float32)


def timed_run(inputs, iters=5):
    """Returns (best_seconds, output). Inputs transferred to device once."""
    import jax

    sharded, in_names, out_names, zero_shapes = _get_exec()
    in_maps = _prep_inputs(**{k: np.asarray(v) for k, v in inputs.items()})
    concat_in = _concat_inputs(in_maps, in_names)
    out = _run_concat(concat_in)  # compile + warm
    mesh = bass2jax.Mesh(np.asarray(jax.devices()[:NCORES]), ("core",))
    shd = jax.sharding.NamedSharding(mesh, bass2jax.PartitionSpec("core"))
    dev_in = [jax.device_put(a, shd) for a in concat_in]
    times = []
    for _ in range(iters):
        zeros = [np.zeros((NCORES * s[0],) + s[1:], d) for s, d in zero_shapes]
        t0 = time.perf_counter()
        r = sharded(*dev_in, *zeros)
        jax.block_until_ready(r)
        times.append(time.perf_counter() - t0)
    return min(times), out.reshape(B, 1)
